# revision 10
# baseline (speedup 1.0000x reference)
"""Trainium2 Bass kernel for nn_BiologicallyInformedBaseline.

Pipeline (matches reference.py):
  pf  = x @ pe_w + pe_b                     # pathway encoder [N, 64]
  pa  = MHA_self(pf)                        # 4 heads, dh=16
  h   = [x, pa]                             # [N, 320]
  h1  = relu(gcn(h,  w1, b1))
  h2  = relu(gcn(h1, w2, b2))
  out = gcn(h2, w3, b3)                     # [N, 64]

Distribution: 8 cores, each owns a 1024-row block of nodes (queries for
attention, dst nodes for the GCN).  The GCN scatter/gather is computed as a
dense matmul against G = (A + I) viewed as small-integer counts stored in
fp8e4 (exact), with the symmetric normalization dinv applied as separate
row/col scalings.  Features move through the chip feature-major
("transposed", [feat, node]) so layer outputs are directly usable as the
next layer's stationary matmul operand.  Three AllGathers (pa, h1, h2)
share per-core blocks between layers.
"""
import sys
import os

sys.path.insert(0, "/opt/trn_rl_repo")

import numpy as np
import ml_dtypes

import concourse.bacc as bacc
import concourse.bass as bass
import concourse.tile as tile
import concourse.mybir as mybir
from concourse.bass_utils import run_bass_kernel_spmd

F32 = mybir.dt.float32
BF16 = mybir.dt.bfloat16
FP8 = mybir.dt.float8e4

NP_BF16 = ml_dtypes.bfloat16
NP_FP8 = ml_dtypes.float8_e4m3

N_NODES = 8192
N_CORES = 8
BLK = N_NODES // N_CORES          # 1024 nodes per core
IN_DIM = 256
HID = 256
OUT_DIM = 64
PD = 64                           # PATH_DIM (attention embed)
NH = 4                            # heads
DH = PD // NH                     # 16
NKC = N_NODES // 128              # 64 key chunks / src chunks
QH = 512                          # query half size (BLK // 2)

_cache = {}


def _bf(x):
    return np.ascontiguousarray(np.asarray(x, dtype=np.float32).astype(NP_BF16))


def _f32(x):
    return np.ascontiguousarray(np.asarray(x, dtype=np.float32))


def _build_program():
    nc = bacc.Bacc("TRN2", target_bir_lowering=False, debug=False,
                   num_devices=N_CORES)

    def inp(name, shape, dt):
        return nc.dram_tensor(name, list(shape), dt, kind="ExternalInput").ap()

    # ---- inputs (shared across cores unless noted) ----
    xT = inp("xT", [2, 128, N_NODES], BF16)          # x.T as 2 chunks of 128 feats
    xsT = inp("xsT", [2, 128, N_NODES], BF16)        # (dinv*x).T
    xblkT = inp("xblkT", [2, 128, BLK], BF16)        # per-core: own block of x.T
    a_blk = inp("a_blk", [N_NODES, BLK], FP8)        # per-core: G[src, own dst block]
    dinv_b = inp("dinv_b", [128, BLK], F32)          # per-core: dinv of own block, bcast 128
    pe_w = inp("pe_w", [2, 128, PD], BF16)           # pathway Linear weight, chunked
    pe_b = inp("pe_b", [PD, 1], F32)
    wq_aug = inp("wq_aug", [PD + 1, 128], BF16)      # [wq.T/4 ; bq.T/4], head-spread
    wk_aug = inp("wk_aug", [PD + 1, 128], BF16)
    wv_aug = inp("wv_aug", [PD + 1, NH * 33], BF16)  # per head: 16 V cols, 16 zero, 1 ones
    wo_sp = inp("wo_sp", [128, PD], BF16)            # out_proj_w.T, rows head-spread
    bo = inp("bo", [PD, 1], F32)
    w1 = inp("w1", [IN_DIM + PD, HID], BF16)
    b1 = inp("b1", [128, 2], F32)
    w2 = inp("w2", [HID, HID], BF16)
    b2 = inp("b2", [128, 2], F32)
    w3 = inp("w3", [HID, OUT_DIM], BF16)
    b3 = inp("b3", [OUT_DIM, 1], F32)
    ind128 = inp("ind128", [128, 128], F32)         # denom row -> head rows indicator

    outT = nc.dram_tensor("outT", [OUT_DIM, BLK], F32, kind="ExternalOutput").ap()

    GRP = [list(range(N_CORES))]

    with tile.TileContext(nc) as tc:
        ctxstack = []
        # ---------- persistent SBUF ----------
        const_pool = tc.alloc_tile_pool(name="consts", bufs=1)
        big_pool = tc.alloc_tile_pool(name="big", bufs=1)

        pe_w_sb = const_pool.tile([128, 2, PD], BF16, tag="pe_w")
        for c in range(2):
            nc.sync.dma_start(pe_w_sb[:, c, :], pe_w[c])
        pe_b_sb = const_pool.tile([PD, 1], F32, tag="pe_b")
        nc.sync.dma_start(pe_b_sb[:], pe_b[:])
        wq_sb = const_pool.tile([PD + 1, 128], BF16, tag="wq")
        nc.sync.dma_start(wq_sb[:], wq_aug[:])
        wk_sb = const_pool.tile([PD + 1, 128], BF16, tag="wk")
        nc.sync.dma_start(wk_sb[:], wk_aug[:])
        wv_sb = const_pool.tile([PD + 1, NH * 33], BF16, tag="wv")
        nc.sync.dma_start(wv_sb[:], wv_aug[:])
        wo_sb = const_pool.tile([128, PD], BF16, tag="wo")
        nc.sync.dma_start(wo_sb[:], wo_sp[:])
        bo_sb = const_pool.tile([PD, 1], F32, tag="bo")
        nc.sync.dma_start(bo_sb[:], bo[:])
        w1_sb = const_pool.tile([128, 2, HID], BF16, tag="w1")
        for c in range(2):
            nc.sync.dma_start(w1_sb[:, c, :], w1[bass.ts(c, 128), :])
        w1p_sb = const_pool.tile([PD, HID], BF16, tag="w1p")
        nc.sync.dma_start(w1p_sb[:], w1[IN_DIM:IN_DIM + PD, :])
        b1_sb = const_pool.tile([128, 2], F32, tag="b1")
        nc.sync.dma_start(b1_sb[:], b1[:])
        w2_sb = const_pool.tile([128, 2, HID], BF16, tag="w2")
        for c in range(2):
            nc.sync.dma_start(w2_sb[:, c, :], w2[bass.ts(c, 128), :])
        b2_sb = const_pool.tile([128, 2], F32, tag="b2")
        nc.sync.dma_start(b2_sb[:], b2[:])
        w3_sb = const_pool.tile([128, 2, OUT_DIM], BF16, tag="w3")
        for c in range(2):
            nc.sync.dma_start(w3_sb[:, c, :], w3[bass.ts(c, 128), :])
        b3_sb = const_pool.tile([OUT_DIM, 1], F32, tag="b3")
        nc.sync.dma_start(b3_sb[:], b3[:])
        ind_sb = const_pool.tile([128, 128], F32, tag="ind128")
        nc.sync.dma_start(ind_sb[:], ind128[:])
        dinv_sb = const_pool.tile([128, BLK], F32, tag="dinv")
        nc.sync.dma_start(dinv_sb[:], dinv_b[:])
        xblk_sb = const_pool.tile([128, 2, BLK], BF16, tag="xblk")
        for c in range(2):
            nc.sync.dma_start(xblk_sb[:, c, :], xblkT[c])

        # x.T lives in a slot later reused by xs.T (phase-disjoint)
        x_sb = big_pool.tile([128, 2, N_NODES], BF16, tag="xbuf")
        for c in range(2):
            nc.sync.dma_start(x_sb[:, c, :], xT[c])

        pf_sb = big_pool.tile([PD + 1, N_NODES], BF16, tag="hw",
                              name="pf_sb",
                              padded_shape=[128, 2 * N_NODES])
        kT_sb = const_pool.tile([128, N_NODES], BF16, tag="kT")
        vaug_sb = const_pool.tile([128, NKC, NH * 33], BF16, tag="vaug")
        pfb_sb = const_pool.tile([PD + 1, BLK], BF16, tag="pfb")
        qT_sb = const_pool.tile([128, BLK], BF16, tag="qT")
        paT_sb = const_pool.tile([PD, BLK], BF16, tag="paT")

        # ---------- phase 1: pathway encoder + K/V/Q projections ----------
        with tc.tile_pool(name="ppsum", bufs=3, space="PSUM") as ppsum:
            nc.vector.memset(pf_sb[PD:PD + 1, :], 1.0)
            nc.vector.memset(pfb_sb[PD:PD + 1, :], 1.0)
            # pf.T [64, 8192]
            for j in range(N_NODES // 512):
                ps = ppsum.tile([PD, 512], F32, tag="pps")
                for c in range(2):
                    nc.tensor.matmul(ps[:], pe_w_sb[:, c, :], x_sb[:, c, bass.ts(j, 512)],
                                     start=(c == 0), stop=(c == 1))
                nc.scalar.activation(pf_sb[0:PD, bass.ts(j, 512)], ps[:],
                                     mybir.ActivationFunctionType.Identity,
                                     bias=pe_b_sb[:], scale=1.0)
            # pf of own block [64, 1024]
            for j in range(BLK // 512):
                ps = ppsum.tile([PD, 512], F32, tag="pps")
                for c in range(2):
                    nc.tensor.matmul(ps[:], pe_w_sb[:, c, :], xblk_sb[:, c, bass.ts(j, 512)],
                                     start=(c == 0), stop=(c == 1))
                nc.scalar.activation(pfb_sb[0:PD, bass.ts(j, 512)], ps[:],
                                     mybir.ActivationFunctionType.Identity,
                                     bias=pe_b_sb[:], scale=1.0)
            # q.T of own block (scale 1/4 folded into wq_aug)
            for j in range(BLK // 512):
                ps = ppsum.tile([128, 512], F32, tag="pps")
                nc.tensor.matmul(ps[:], wq_sb[:], pfb_sb[:, bass.ts(j, 512)],
                                 start=True, stop=True)
                nc.vector.tensor_copy(qT_sb[:, bass.ts(j, 512)], ps[:])
            # k.T of all nodes
            for j in range(N_NODES // 512):
                ps = ppsum.tile([128, 512], F32, tag="pps")
                nc.tensor.matmul(ps[:], wk_sb[:], pf_sb[:, bass.ts(j, 512)],
                                 start=True, stop=True)
                if j % 2 == 0:
                    nc.vector.tensor_copy(kT_sb[:, bass.ts(j, 512)], ps[:])
                else:
                    nc.scalar.copy(kT_sb[:, bass.ts(j, 512)], ps[:])
            # V (node-major), head-spread with ones column per head
            for s in range(NKC):
                ps = ppsum.tile([128, NH * 33], F32, tag="pps")
                nc.tensor.matmul(ps[:], pf_sb[:, bass.ts(s, 128)], wv_sb[:],
                                 start=True, stop=True)
                if s % 2 == 0:
                    nc.vector.tensor_copy(vaug_sb[:, s, :], ps[:])
                else:
                    nc.scalar.copy(vaug_sb[:, s, :], ps[:])

        # ---------- phase 2: attention over own query block ----------
        with tc.tile_pool(name="spsum", bufs=2, space="PSUM") as spsum, \
             tc.tile_pool(name="avpsum", bufs=1, space="PSUM") as avpsum, \
             tc.tile_pool(name="stile", bufs=4) as stile, \
             tc.tile_pool(name="atmp", bufs=2) as atmp:
            for half in range(2):
                q0 = half * QH
                avs = [avpsum.tile([33, QH], F32, tag=f"av{h}", name=f"av{h}")
                       for h in range(NH)]
                for kc in range(NKC):
                    es = []
                    for hp in range(2):          # head pairs (0,1) and (2,3)
                        sp = spsum.tile([128, 2 * QH], F32, tag="sps")
                        for i in range(2):
                            h = hp * 2 + i
                            nc.tensor.matmul(
                                sp[:, bass.ts(i, QH)],
                                kT_sb[h * 32:h * 32 + DH, bass.ts(kc, 128)],
                                qT_sb[h * 32:h * 32 + DH, q0:q0 + QH],
                                start=True, stop=True,
                                tile_position=(h * 32, 0))
                        ss = stile.tile([128, 2 * QH], BF16, tag="ss")
                        nc.scalar.activation(ss[:], sp[:],
                                             mybir.ActivationFunctionType.Exp)
                        es.append(ss)
                    for hp in range(2):
                        for i in range(2):
                            h = hp * 2 + i
                            nc.tensor.matmul(
                                avs[h][:],
                                vaug_sb[:, kc, h * 33:(h + 1) * 33],
                                es[hp][:, bass.ts(i, QH)],
                                start=(kc == 0), stop=(kc == NKC - 1))
                # normalize: recip of denominators (row 32 of each av tile),
                # broadcast to head rows via an indicator matmul
                r_sp = atmp.tile([128, QH], F32, tag="rsp")
                nc.vector.memset(r_sp[:], 0.0)
                for h in range(NH):
                    nc.vector.reciprocal(r_sp[h * 32:h * 32 + 1, :],
                                         avs[h][32:33, :])
                rb = spsum.tile([128, QH], F32, tag="sps", name="rb_ps")
                nc.tensor.matmul(rb[:], ind_sb[:], r_sp[:], start=True, stop=True)
                rbs = atmp.tile([128, QH], F32, tag="rbs")
                nc.vector.tensor_copy(rbs[:], rb[:])
                at_sp = atmp.tile([128, QH], BF16, tag="atsp")
                nc.vector.memset(at_sp[:], 0.0)
                for h in range(NH):
                    nc.vector.tensor_mul(at_sp[h * 32:h * 32 + DH, :],
                                         avs[h][0:DH, :],
                                         rbs[h * 32:h * 32 + DH, :])
                # out projection + bias, then dinv scaling for the GCN concat
                pp = spsum.tile([PD, QH], F32, tag="sps", name="pp_ps")
                nc.tensor.matmul(pp[:], wo_sb[:], at_sp[:], start=True, stop=True)
                pt = atmp.tile([PD, QH], F32, tag="pt")
                nc.scalar.activation(pt[:], pp[:],
                                     mybir.ActivationFunctionType.Identity,
                                     bias=bo_sb[:], scale=1.0)
                nc.vector.tensor_mul(paT_sb[:, q0:q0 + QH], pt[:],
                                     dinv_sb[0:PD, q0:q0 + QH])

        # ---------- phase 3: gather pa across cores ----------
        with tc.tile_pool(name="dram", bufs=1, space="DRAM") as dram:
            pa_in = dram.tile([PD, BLK], BF16, tag="pa_in")
            nc.sync.dma_start(pa_in[:], paT_sb[:])
            pa_all = dram.tile([N_CORES, PD, BLK], BF16, tag="pa_all",
                               addr_space="Shared")
            nc.gpsimd.collective_compute(
                "AllGather", mybir.AluOpType.bypass, replica_groups=GRP,
                ins=[pa_in.opt()], outs=[pa_all.opt()])
            paf_sb = big_pool.tile([PD, N_CORES, BLK], BF16, tag="hgat",
                                   name="paf_sb",
                                   padded_shape=[128, N_CORES, 2 * N_NODES // N_CORES])
            for c in range(N_CORES):
                nc.sync.dma_start(paf_sb[:, c, :], pa_all[c])

            # xs.T reuses the x.T slot (pf/K/V/Q all consumed x.T already)
            xs_sb = big_pool.tile([128, 2, N_NODES], BF16, tag="xbuf")
            for c in range(2):
                nc.sync.dma_start(xs_sb[:, c, :], xsT[c])

            hw_sb = big_pool.tile([128, NKC, HID], BF16, tag="hw",
                                  name="hw_sb",
                                  padded_shape=[128, NKC, 2 * N_NODES // NKC])
            hgat_sb = big_pool.tile([128, 2, N_NODES], BF16, tag="hgat",
                                    name="hgat_sb",
                                    padded_shape=[128, 2, N_NODES])

            # ---------- HW1 = H1 @ w1  (feature-major H1 = [xs.T ; pa.T]) ----------
            with tc.tile_pool(name="hwpsum", bufs=3, space="PSUM") as hwpsum:
                for s in range(NKC):
                    ps = hwpsum.tile([128, HID], F32, tag="hwps")
                    nc.tensor.matmul(ps[:], xs_sb[:, 0, bass.ts(s, 128)],
                                     w1_sb[:, 0, :], start=True, stop=False)
                    nc.tensor.matmul(ps[:], xs_sb[:, 1, bass.ts(s, 128)],
                                     w1_sb[:, 1, :], start=False, stop=False)
                    nc.tensor.matmul(
                        ps[:],
                        paf_sb[:, s // 8, (s % 8) * 128:(s % 8) * 128 + 128],
                        w1p_sb[:], start=False, stop=True)
                    if s % 2 == 0:
                        nc.vector.tensor_copy(hw_sb[:, s, :], ps[:])
                    else:
                        nc.scalar.copy(hw_sb[:, s, :], ps[:])

            def gcn_accumulate(apool, gpsum, n_feat, layer_tag):
                """psum[f][d] += sum_s HW[s, f*128:...]^T-contracted A chunks."""
                nf = (n_feat + 127) // 128
                ps = [[gpsum.tile([min(128, n_feat), 512], F32,
                                  tag=f"g{layer_tag}{f}{d}",
                                  name=f"gps_{layer_tag}{f}{d}")
                       for d in range(2)] for f in range(nf)]
                for s in range(NKC):
                    a_t = apool.tile([128, BLK], FP8, tag="achunk")
                    nc.sync.dma_start(a_t[:], a_blk[bass.ts(s, 128), :])
                    for f in range(nf):
                        for d in range(2):
                            nc.tensor.matmul(
                                ps[f][d][:],
                                hw_sb[:, s, f * 128:f * 128 + min(128, n_feat)],
                                a_t[:, bass.ts(d, 512)],
                                start=(s == 0), stop=(s == NKC - 1))
                return ps

            def gcn_finish_relu(gps, b_sb, out_sb, tpool):
                """out = dinv * relu(dinv * psum + b), bf16, feature-major."""
                for f in range(2):
                    for d in range(2):
                        dsl = dinv_sb[:, bass.ts(d, 512)]
                        t1 = tpool.tile([128, 512], F32, tag="t1")
                        nc.vector.tensor_mul(t1[:], gps[f][d][:], dsl)
                        t2 = tpool.tile([128, 512], F32, tag="t2")
                        nc.scalar.activation(t2[:], t1[:],
                                             mybir.ActivationFunctionType.Relu,
                                             bias=b_sb[:, f:f + 1], scale=1.0)
                        nc.vector.tensor_mul(out_sb[:, f, bass.ts(d, 512)],
                                             t2[:], dsl)

            def allgather_h(tag, src_sb):
                h_in = dram.tile([2, 128, BLK], BF16, tag=f"hin{tag}")
                for f in range(2):
                    nc.sync.dma_start(h_in[f], src_sb[:, f, :])
                h_all = dram.tile([N_CORES, 2, 128, BLK], BF16, tag=f"hall{tag}",
                                  addr_space="Shared")
                nc.gpsimd.collective_compute(
                    "AllGather", mybir.AluOpType.bypass, replica_groups=GRP,
                    ins=[h_in.opt()], outs=[h_all.opt()])
                for c in range(N_CORES):
                    for f in range(2):
                        nc.sync.dma_start(
                            hgat_sb[:, f, c * BLK:(c + 1) * BLK], h_all[c, f])

            def compute_hw(w_sb, n_out):
                with tc.tile_pool(name="hwpsum2", bufs=3, space="PSUM") as hp:
                    for s in range(NKC):
                        ps = hp.tile([128, n_out], F32, tag="hwps2")
                        nc.tensor.matmul(ps[:], hgat_sb[:, 0, bass.ts(s, 128)],
                                         w_sb[:, 0, :], start=True, stop=False)
                        nc.tensor.matmul(ps[:], hgat_sb[:, 1, bass.ts(s, 128)],
                                         w_sb[:, 1, :], start=False, stop=True)
                        if s % 2 == 0:
                            nc.vector.tensor_copy(hw_sb[:, s, 0:n_out], ps[:])
                        else:
                            nc.scalar.copy(hw_sb[:, s, 0:n_out], ps[:])

            with tc.tile_pool(name="apool", bufs=4) as apool, \
                 tc.tile_pool(name="gtmp", bufs=2) as gtmp, \
                 tc.tile_pool(name="hblk", bufs=2) as hblk:
                # ----- layer 1 -----
                with tc.tile_pool(name="gps1", bufs=1, space="PSUM") as gp1:
                    gcn_ps = gcn_accumulate(apool, gp1, HID, "a")
                    h1_sb = hblk.tile([128, 2, BLK], BF16, tag="hout")
                    gcn_finish_relu(gcn_ps, b1_sb, h1_sb, gtmp)
                allgather_h("1", h1_sb)
                compute_hw(w2_sb, HID)
                # ----- layer 2 -----
                with tc.tile_pool(name="gps2", bufs=1, space="PSUM") as gp2:
                    gcn_ps = gcn_accumulate(apool, gp2, HID, "b")
                    h2_sb = hblk.tile([128, 2, BLK], BF16, tag="hout")
                    gcn_finish_relu(gcn_ps, b2_sb, h2_sb, gtmp)
                allgather_h("2", h2_sb)
                compute_hw(w3_sb, OUT_DIM)
                # ----- layer 3 (no relu, f32 out) -----
                with tc.tile_pool(name="gps3", bufs=1, space="PSUM") as gp3:
                    ps3 = [gp3.tile([OUT_DIM, 512], F32, tag=f"g3{d}",
                                    name=f"gps3_{d}")
                           for d in range(2)]
                    for s in range(NKC):
                        a_t = apool.tile([128, BLK], FP8, tag="achunk")
                        nc.sync.dma_start(a_t[:], a_blk[bass.ts(s, 128), :])
                        for d in range(2):
                            nc.tensor.matmul(ps3[d][:], hw_sb[:, s, 0:OUT_DIM],
                                             a_t[:, bass.ts(d, 512)],
                                             start=(s == 0), stop=(s == NKC - 1))
                    o_sb = hblk.tile([OUT_DIM, BLK], F32, tag="osb", bufs=1)
                    for d in range(2):
                        t1 = gtmp.tile([OUT_DIM, 512], F32, tag="t3")
                        nc.vector.tensor_mul(t1[:], ps3[d][:],
                                             dinv_sb[0:OUT_DIM, bass.ts(d, 512)])
                        nc.scalar.activation(o_sb[:, bass.ts(d, 512)], t1[:],
                                             mybir.ActivationFunctionType.Identity,
                                             bias=b3_sb[:], scale=1.0)
                    nc.sync.dma_start(outT[:], o_sb[:])

        big_pool.release()
        const_pool.release()

    nc.compile()
    return nc


def _preprocess(x, edge_index, pe_w, pe_b, in_proj_w, in_proj_b,
                out_proj_w, out_proj_b, w1, b1, w2, b2, w3, b3):
    """Host-side sharding + weight folding. Returns per-core input maps."""
    x = _f32(x)
    src = np.asarray(edge_index[0], dtype=np.int64)
    dst = np.asarray(edge_index[1], dtype=np.int64)

    # G[src, dst] = edge multiplicity + self loops (small exact ints)
    G = np.zeros((N_NODES, N_NODES), dtype=np.float32)
    np.add.at(G, (src, dst), 1.0)
    idx = np.arange(N_NODES)
    G[idx, idx] += 1.0
    deg = G.sum(axis=0)
    dinv = (1.0 / np.sqrt(deg)).astype(np.float32)
    G8 = G.astype(NP_FP8)

    xT = _bf(x.T).reshape(2, 128, N_NODES)
    xsT = _bf((x * dinv[:, None]).T).reshape(2, 128, N_NODES)

    ipw = _f32(in_proj_w)
    ipb = _f32(in_proj_b)

    def aug_spread(w, b):  # [65, 128]: head h -> cols h*32 .. h*32+16
        out = np.zeros((PD + 1, 128), dtype=np.float32)
        for h in range(NH):
            out[0:PD, h * 32:h * 32 + DH] = w[h * DH:(h + 1) * DH].T
            out[PD, h * 32:h * 32 + DH] = b[h * DH:(h + 1) * DH]
        return _bf(out)

    wq_aug = aug_spread(ipw[0:PD] / 4.0, ipb[0:PD] / 4.0)
    wk_aug = aug_spread(ipw[PD:2 * PD], ipb[PD:2 * PD])
    wv = ipw[2 * PD:3 * PD]
    bv = ipb[2 * PD:3 * PD]
    # per head h (33 cols): cols 0..15 = wv_h.T (+bias row), 16..31 = 0,
    # col 32 = ones-row trick -> AV psum row 32 = softmax denominator
    wv_aug = np.zeros((PD + 1, NH * 33), dtype=np.float32)
    for h in range(NH):
        wv_aug[0:PD, h * 33:h * 33 + DH] = wv[h * DH:(h + 1) * DH].T
        wv_aug[PD, h * 33:h * 33 + DH] = bv[h * DH:(h + 1) * DH]
        wv_aug[PD, h * 33 + 32] = 1.0
    wv_aug = _bf(wv_aug)

    # wo spread: rows h*32..h*32+16 = out_proj_w.T rows h*16..h*16+16
    wo_sp = np.zeros((128, PD), dtype=np.float32)
    woT = np.asarray(out_proj_w, dtype=np.float32).T
    for h in range(NH):
        wo_sp[h * 32:h * 32 + DH, :] = woT[h * DH:(h + 1) * DH, :]
    wo_sp = _bf(wo_sp)

    # indicator: rb[f, q] = r_sp[32*(f//32), q] for data rows, 0 for pad rows
    ind128 = np.zeros((128, 128), dtype=np.float32)
    for f in range(128):
        if f % 32 < DH:
            ind128[(f // 32) * 32, f] = 1.0

    shared = {
        "xT": xT, "xsT": xsT,
        "pe_w": _bf(pe_w).reshape(2, 128, PD),
        "pe_b": _f32(pe_b).reshape(PD, 1),
        "wq_aug": wq_aug, "wk_aug": wk_aug, "wv_aug": wv_aug,
        "wo_sp": wo_sp,
        "bo": _f32(out_proj_b).reshape(PD, 1),
        "w1": _bf(w1), "b1": _f32(b1).reshape(2, 128).T.copy(),
        "w2": _bf(w2), "b2": _f32(b2).reshape(2, 128).T.copy(),
        "w3": _bf(w3), "b3": _f32(b3).reshape(OUT_DIM, 1),
        "ind128": ind128,
    }
    in_maps = []
    for c in range(N_CORES):
        lo, hi = c * BLK, (c + 1) * BLK
        m = dict(shared)
        m["xblkT"] = np.ascontiguousarray(
            xT.reshape(IN_DIM, N_NODES)[:, lo:hi]).reshape(2, 128, BLK)
        m["a_blk"] = np.ascontiguousarray(G8[:, lo:hi])
        m["dinv_b"] = np.ascontiguousarray(
            np.broadcast_to(dinv[lo:hi][None, :], (128, BLK)))
        in_maps.append(m)
    return in_maps


def kernel(**inputs):
    if "nc" not in _cache:
        _cache["nc"] = _build_program()
    nc = _cache["nc"]
    in_maps = _preprocess(**inputs)
    res = run_bass_kernel_spmd(nc, in_maps, list(range(N_CORES)))
    out = np.concatenate(
        [np.asarray(res.results[c]["outT"], dtype=np.float32).T
         for c in range(N_CORES)], axis=0)
    return out


# revision 12
# speedup vs baseline: 5622.2459x; 5622.2459x over previous
"""Trainium2 Bass kernel for nn_BiologicallyInformedBaseline.

Pipeline (matches reference.py):
  pf  = x @ pe_w + pe_b                     # pathway encoder [N, 64]
  pa  = MHA_self(pf)                        # 4 heads, dh=16
  h   = [x, pa]                             # [N, 320]
  h1  = relu(gcn(h,  w1, b1))
  h2  = relu(gcn(h1, w2, b2))
  out = gcn(h2, w3, b3)                     # [N, 64]

Distribution: 8 cores, each owns a 1024-row block of nodes (queries for
attention, dst nodes for the GCN).  The GCN scatter/gather is computed as a
dense matmul against G = (A + I) viewed as small-integer counts stored in
fp8e4 (exact), with the symmetric normalization dinv applied as separate
row/col scalings.  Features move through the chip feature-major
("transposed", [feat, node]) so layer outputs are directly usable as the
next layer's stationary matmul operand.  Three AllGathers (pa, h1, h2)
share per-core blocks between layers.
"""
import sys
import os

sys.path.insert(0, "/opt/trn_rl_repo")

import numpy as np
import ml_dtypes

import concourse.bacc as bacc
import concourse.bass as bass
import concourse.tile as tile
import concourse.mybir as mybir
from concourse.bass_utils import run_bass_kernel_spmd

F32 = mybir.dt.float32
BF16 = mybir.dt.bfloat16
FP8 = mybir.dt.float8e4

NP_BF16 = ml_dtypes.bfloat16
NP_FP8 = ml_dtypes.float8_e4m3

N_NODES = 8192
N_CORES = 8
BLK = N_NODES // N_CORES          # 1024 nodes per core
IN_DIM = 256
HID = 256
OUT_DIM = 64
PD = 64                           # PATH_DIM (attention embed)
NH = 4                            # heads
DH = PD // NH                     # 16
NKC = N_NODES // 128              # 64 key chunks / src chunks
QH = 512                          # query half size (BLK // 2)

_cache = {}


def _bf(x):
    return np.ascontiguousarray(np.asarray(x, dtype=np.float32).astype(NP_BF16))


def _f32(x):
    return np.ascontiguousarray(np.asarray(x, dtype=np.float32))


def _build_program(sim=False):
    """sim=True builds a single-core variant (collectives replaced by local
    DMA copies with equivalent traffic) for TimelineSim cost analysis."""
    nc = bacc.Bacc("TRN2", target_bir_lowering=False, debug=False,
                   num_devices=1 if sim else N_CORES)

    def inp(name, shape, dt):
        return nc.dram_tensor(name, list(shape), dt, kind="ExternalInput").ap()

    # ---- inputs (shared across cores unless noted) ----
    xT = inp("xT", [2, 128, N_NODES], BF16)          # x.T as 2 chunks of 128 feats
    xsT = inp("xsT", [2, 128, N_NODES], BF16)        # (dinv*x).T
    xblkT = inp("xblkT", [2, 128, BLK], BF16)        # per-core: own block of x.T
    a_blk = inp("a_blk", [N_NODES, BLK], FP8)        # per-core: G[src, own dst block]
    dinv_b = inp("dinv_b", [128, BLK], F32)          # per-core: dinv of own block, bcast 128
    pe_w = inp("pe_w", [2, 128, PD], BF16)           # pathway Linear weight, chunked
    pe_b = inp("pe_b", [PD, 1], F32)
    wq_aug = inp("wq_aug", [PD + 1, 128], BF16)      # [wq.T/4 ; bq.T/4], head-spread
    wk_aug = inp("wk_aug", [PD + 1, 128], BF16)
    wv_aug = inp("wv_aug", [PD + 1, NH * 33], BF16)  # per head: 16 V cols, 16 zero, 1 ones
    wo_sp = inp("wo_sp", [128, PD], BF16)            # out_proj_w.T, rows head-spread
    bo = inp("bo", [PD, 1], F32)
    w1 = inp("w1", [IN_DIM + PD, HID], BF16)
    b1 = inp("b1", [128, 2], F32)
    w2 = inp("w2", [HID, HID], BF16)
    b2 = inp("b2", [128, 2], F32)
    w3 = inp("w3", [HID, OUT_DIM], BF16)
    b3 = inp("b3", [OUT_DIM, 1], F32)
    ind128 = inp("ind128", [128, 128], F32)         # denom row -> head rows indicator

    outT = nc.dram_tensor("outT", [OUT_DIM, BLK], F32, kind="ExternalOutput").ap()

    GRP = [list(range(N_CORES))]

    with tile.TileContext(nc) as tc:
        ctxstack = []
        # ---------- persistent SBUF ----------
        const_pool = tc.alloc_tile_pool(name="consts", bufs=1)
        big_pool = tc.alloc_tile_pool(name="big", bufs=1)

        pe_w_sb = const_pool.tile([128, 2, PD], BF16, tag="pe_w")
        for c in range(2):
            nc.sync.dma_start(pe_w_sb[:, c, :], pe_w[c])
        pe_b_sb = const_pool.tile([PD, 1], F32, tag="pe_b")
        nc.sync.dma_start(pe_b_sb[:], pe_b[:])
        wq_sb = const_pool.tile([PD + 1, 128], BF16, tag="wq")
        nc.sync.dma_start(wq_sb[:], wq_aug[:])
        wk_sb = const_pool.tile([PD + 1, 128], BF16, tag="wk")
        nc.sync.dma_start(wk_sb[:], wk_aug[:])
        wv_sb = const_pool.tile([PD + 1, NH * 33], BF16, tag="wv")
        nc.sync.dma_start(wv_sb[:], wv_aug[:])
        wo_sb = const_pool.tile([128, PD], BF16, tag="wo")
        nc.sync.dma_start(wo_sb[:], wo_sp[:])
        bo_sb = const_pool.tile([PD, 1], F32, tag="bo")
        nc.sync.dma_start(bo_sb[:], bo[:])
        w1_sb = const_pool.tile([128, 2, HID], BF16, tag="w1")
        for c in range(2):
            nc.sync.dma_start(w1_sb[:, c, :], w1[bass.ts(c, 128), :])
        w1p_sb = const_pool.tile([PD, HID], BF16, tag="w1p")
        nc.sync.dma_start(w1p_sb[:], w1[IN_DIM:IN_DIM + PD, :])
        b1_sb = const_pool.tile([128, 2], F32, tag="b1")
        nc.sync.dma_start(b1_sb[:], b1[:])
        w2_sb = const_pool.tile([128, 2, HID], BF16, tag="w2")
        for c in range(2):
            nc.sync.dma_start(w2_sb[:, c, :], w2[bass.ts(c, 128), :])
        b2_sb = const_pool.tile([128, 2], F32, tag="b2")
        nc.sync.dma_start(b2_sb[:], b2[:])
        w3_sb = const_pool.tile([128, 2, OUT_DIM], BF16, tag="w3")
        for c in range(2):
            nc.sync.dma_start(w3_sb[:, c, :], w3[bass.ts(c, 128), :])
        b3_sb = const_pool.tile([OUT_DIM, 1], F32, tag="b3")
        nc.sync.dma_start(b3_sb[:], b3[:])
        ind_sb = const_pool.tile([128, 128], F32, tag="ind128")
        nc.sync.dma_start(ind_sb[:], ind128[:])
        dinv_sb = const_pool.tile([128, BLK], F32, tag="dinv")
        nc.sync.dma_start(dinv_sb[:], dinv_b[:])
        xblk_sb = const_pool.tile([128, 2, BLK], BF16, tag="xblk")
        for c in range(2):
            nc.sync.dma_start(xblk_sb[:, c, :], xblkT[c])

        # x.T lives in a slot later reused by xs.T (phase-disjoint)
        x_sb = big_pool.tile([128, 2, N_NODES], BF16, tag="xbuf")
        for c in range(2):
            nc.sync.dma_start(x_sb[:, c, :], xT[c])

        pf_sb = big_pool.tile([PD + 1, N_NODES], BF16, tag="hw",
                              name="pf_sb",
                              padded_shape=[128, 2 * N_NODES])
        kT_sb = const_pool.tile([128, N_NODES], BF16, tag="kT")
        vaug_sb = const_pool.tile([128, NKC, NH * 33], BF16, tag="vaug")
        pfb_sb = const_pool.tile([PD + 1, BLK], BF16, tag="pfb")
        qT_sb = const_pool.tile([128, BLK], BF16, tag="qT")
        paT_sb = const_pool.tile([PD, BLK], BF16, tag="paT")

        # ---------- phase 1: pathway encoder + K/V/Q projections ----------
        with tc.tile_pool(name="ppsum", bufs=3, space="PSUM") as ppsum:
            nc.vector.memset(pf_sb[PD:PD + 1, :], 1.0)
            nc.vector.memset(pfb_sb[PD:PD + 1, :], 1.0)
            # pf.T [64, 8192]
            for j in range(N_NODES // 512):
                ps = ppsum.tile([PD, 512], F32, tag="pps")
                for c in range(2):
                    nc.tensor.matmul(ps[:], pe_w_sb[:, c, :], x_sb[:, c, bass.ts(j, 512)],
                                     start=(c == 0), stop=(c == 1))
                nc.scalar.activation(pf_sb[0:PD, bass.ts(j, 512)], ps[:],
                                     mybir.ActivationFunctionType.Identity,
                                     bias=pe_b_sb[:], scale=1.0)
            # pf of own block [64, 1024]
            for j in range(BLK // 512):
                ps = ppsum.tile([PD, 512], F32, tag="pps")
                for c in range(2):
                    nc.tensor.matmul(ps[:], pe_w_sb[:, c, :], xblk_sb[:, c, bass.ts(j, 512)],
                                     start=(c == 0), stop=(c == 1))
                nc.scalar.activation(pfb_sb[0:PD, bass.ts(j, 512)], ps[:],
                                     mybir.ActivationFunctionType.Identity,
                                     bias=pe_b_sb[:], scale=1.0)
            # q.T of own block (scale 1/4 folded into wq_aug)
            for j in range(BLK // 512):
                ps = ppsum.tile([128, 512], F32, tag="pps")
                nc.tensor.matmul(ps[:], wq_sb[:], pfb_sb[:, bass.ts(j, 512)],
                                 start=True, stop=True)
                nc.vector.tensor_copy(qT_sb[:, bass.ts(j, 512)], ps[:])
            # k.T of all nodes
            for j in range(N_NODES // 512):
                ps = ppsum.tile([128, 512], F32, tag="pps")
                nc.tensor.matmul(ps[:], wk_sb[:], pf_sb[:, bass.ts(j, 512)],
                                 start=True, stop=True)
                if j % 2 == 0:
                    nc.vector.tensor_copy(kT_sb[:, bass.ts(j, 512)], ps[:])
                else:
                    nc.scalar.copy(kT_sb[:, bass.ts(j, 512)], ps[:])
            # V (node-major), head-spread with ones column per head
            for s in range(NKC):
                ps = ppsum.tile([128, NH * 33], F32, tag="pps")
                nc.tensor.matmul(ps[:], pf_sb[:, bass.ts(s, 128)], wv_sb[:],
                                 start=True, stop=True)
                if s % 2 == 0:
                    nc.vector.tensor_copy(vaug_sb[:, s, :], ps[:])
                else:
                    nc.scalar.copy(vaug_sb[:, s, :], ps[:])

        # ---------- phase 2: attention over own query block ----------
        # scores: 4 heads row-tiled (K=16 strips at h*32) -> concurrent on PE
        # AV: heads packed in pairs at out bases {0, 64} -> 2-way col groups
        # exp pipelined: expA(kc) || PE[sc01(kc+1), av01(kc)] || expB(kc) ...
        with tc.tile_pool(name="spsum", bufs=1, space="PSUM") as spsum, \
             tc.tile_pool(name="avpsum", bufs=1, space="PSUM") as avpsum, \
             tc.tile_pool(name="stile", bufs=2) as stile, \
             tc.tile_pool(name="atmp", bufs=2) as atmp:
            for half in range(2):
                q0 = half * QH
                avt = [avpsum.tile([128, QH], F32, tag=f"av{j}", name=f"av{j}")
                       for j in range(2)]

                def scores_pair(hp, kc):
                    sp = spsum.tile([128, 2 * QH], F32, tag=f"sps{hp}",
                                    name=f"sps{hp}", bufs=1)
                    for i in range(2):
                        h = hp * 2 + i
                        nc.tensor.matmul(
                            sp[:, bass.ts(i, QH)],
                            kT_sb[h * 32:h * 32 + DH, bass.ts(kc, 128)],
                            qT_sb[h * 32:h * 32 + DH, q0:q0 + QH],
                            start=True, stop=True,
                            tile_position=(h * 32, 0))
                    ss = stile.tile([128, 2 * QH], BF16, tag=f"ss{hp}",
                                    name=f"ss{hp}", bufs=2)
                    nc.scalar.activation(ss[:], sp[:],
                                         mybir.ActivationFunctionType.Exp)
                    return ss

                def av_pair(hp, kc, ss):
                    for i in range(2):
                        h = hp * 2 + i
                        nc.tensor.matmul(
                            avt[hp][i * 64:i * 64 + 33, :],
                            vaug_sb[:, kc, h * 33:(h + 1) * 33],
                            ss[:, bass.ts(i, QH)],
                            start=(kc == 0), stop=(kc == NKC - 1),
                            tile_position=(0, i * 64))

                prev = None
                for kc in range(NKC):
                    ssA = scores_pair(0, kc)
                    if prev is not None:
                        av_pair(0, kc - 1, prev[0])
                    ssB = scores_pair(1, kc)
                    if prev is not None:
                        av_pair(1, kc - 1, prev[1])
                    prev = (ssA, ssB)
                av_pair(0, NKC - 1, prev[0])
                av_pair(1, NKC - 1, prev[1])

                # normalize: head h data rows at avt[h//2][(h%2)*64 ..+16],
                # denominator at avt[h//2][(h%2)*64 + 32]
                r_sp = atmp.tile([128, QH], F32, tag="rsp")
                nc.vector.memset(r_sp[:], 0.0)
                for h in range(NH):
                    b = (h % 2) * 64
                    nc.vector.reciprocal(r_sp[h * 32:h * 32 + 1, :],
                                         avt[h // 2][b + 32:b + 33, :])
                rb = spsum.tile([128, QH], F32, tag="sps0", name="rb_ps")
                nc.tensor.matmul(rb[:], ind_sb[:], r_sp[:], start=True, stop=True)
                rbs = atmp.tile([128, QH], F32, tag="rbs")
                nc.vector.tensor_copy(rbs[:], rb[:])
                at_sp = atmp.tile([128, QH], BF16, tag="atsp")
                nc.vector.memset(at_sp[:], 0.0)
                for h in range(NH):
                    b = (h % 2) * 64
                    nc.vector.tensor_mul(at_sp[h * 32:h * 32 + DH, :],
                                         avt[h // 2][b:b + DH, :],
                                         rbs[h * 32:h * 32 + DH, :])
                # out projection + bias, then dinv scaling for the GCN concat
                pp = spsum.tile([PD, QH], F32, tag="sps1", name="pp_ps")
                nc.tensor.matmul(pp[:], wo_sb[:], at_sp[:], start=True, stop=True)
                pt = atmp.tile([PD, QH], F32, tag="pt")
                nc.scalar.activation(pt[:], pp[:],
                                     mybir.ActivationFunctionType.Identity,
                                     bias=bo_sb[:], scale=1.0)
                nc.vector.tensor_mul(paT_sb[:, q0:q0 + QH], pt[:],
                                     dinv_sb[0:PD, q0:q0 + QH])

        # ---------- phase 3: gather pa across cores ----------
        with tc.tile_pool(name="dram", bufs=1, space="DRAM") as dram:
            pa_in = dram.tile([PD, BLK], BF16, tag="pa_in")
            nc.sync.dma_start(pa_in[:], paT_sb[:])
            pa_all = dram.tile([N_CORES, PD, BLK], BF16, tag="pa_all",
                               addr_space="Local" if sim else "Shared")
            if sim:
                for c in range(N_CORES):
                    nc.sync.dma_start(pa_all[c], pa_in[:])
            else:
                nc.gpsimd.collective_compute(
                    "AllGather", mybir.AluOpType.bypass, replica_groups=GRP,
                    ins=[pa_in.opt()], outs=[pa_all.opt()])
            paf_sb = big_pool.tile([PD, N_CORES, BLK], BF16, tag="hgat",
                                   name="paf_sb",
                                   padded_shape=[128, N_CORES, 2 * N_NODES // N_CORES])
            for c in range(N_CORES):
                nc.sync.dma_start(paf_sb[:, c, :], pa_all[c])

            # xs.T reuses the x.T slot (pf/K/V/Q all consumed x.T already)
            xs_sb = big_pool.tile([128, 2, N_NODES], BF16, tag="xbuf")
            for c in range(2):
                nc.sync.dma_start(xs_sb[:, c, :], xsT[c])

            hw_sb = big_pool.tile([128, NKC, HID], BF16, tag="hw",
                                  name="hw_sb",
                                  padded_shape=[128, NKC, 2 * N_NODES // NKC])
            hgat_sb = big_pool.tile([128, 2, N_NODES], BF16, tag="hgat",
                                    name="hgat_sb",
                                    padded_shape=[128, 2, N_NODES])

            # ---------- HW1 = H1 @ w1  (feature-major H1 = [xs.T ; pa.T]) ----------
            with tc.tile_pool(name="hwpsum", bufs=3, space="PSUM") as hwpsum:
                for s in range(NKC):
                    ps = hwpsum.tile([128, HID], F32, tag="hwps")
                    nc.tensor.matmul(ps[:], xs_sb[:, 0, bass.ts(s, 128)],
                                     w1_sb[:, 0, :], start=True, stop=False)
                    nc.tensor.matmul(ps[:], xs_sb[:, 1, bass.ts(s, 128)],
                                     w1_sb[:, 1, :], start=False, stop=False)
                    nc.tensor.matmul(
                        ps[:],
                        paf_sb[:, s // 8, (s % 8) * 128:(s % 8) * 128 + 128],
                        w1p_sb[:], start=False, stop=True)
                    if s % 2 == 0:
                        nc.vector.tensor_copy(hw_sb[:, s, :], ps[:])
                    else:
                        nc.scalar.copy(hw_sb[:, s, :], ps[:])

            def gcn_accumulate(apool, gpsum, n_feat, layer_tag):
                """psum[f][d] += sum_s HW[s, f*128:...]^T-contracted A chunks."""
                nf = (n_feat + 127) // 128
                ps = [[gpsum.tile([min(128, n_feat), 512], F32,
                                  tag=f"g{layer_tag}{f}{d}",
                                  name=f"gps_{layer_tag}{f}{d}")
                       for d in range(2)] for f in range(nf)]
                ab = a_blk.rearrange("(g s p) d -> g p s d", s=4, p=128)
                for g in range(NKC // 4):
                    a_t = apool.tile([128, 4, BLK], FP8, tag="achunk")
                    nc.sync.dma_start(a_t[:], ab[g])
                    for si in range(4):
                        s = g * 4 + si
                        for f in range(nf):
                            for d in range(2):
                                nc.tensor.matmul(
                                    ps[f][d][:],
                                    hw_sb[:, s, f * 128:f * 128 + min(128, n_feat)],
                                    a_t[:, si, bass.ts(d, 512)],
                                    start=(s == 0), stop=(s == NKC - 1))
                return ps

            def gcn_finish_relu(gps, b_sb, out_sb, tpool):
                """out = dinv * relu(dinv * psum + b), bf16, feature-major."""
                for f in range(2):
                    for d in range(2):
                        dsl = dinv_sb[:, bass.ts(d, 512)]
                        t1 = tpool.tile([128, 512], F32, tag="t1")
                        nc.vector.tensor_mul(t1[:], gps[f][d][:], dsl)
                        t2 = tpool.tile([128, 512], F32, tag="t2")
                        nc.scalar.activation(t2[:], t1[:],
                                             mybir.ActivationFunctionType.Relu,
                                             bias=b_sb[:, f:f + 1], scale=1.0)
                        nc.vector.tensor_mul(out_sb[:, f, bass.ts(d, 512)],
                                             t2[:], dsl)

            def allgather_h(tag, src_sb):
                h_in = dram.tile([2, 128, BLK], BF16, tag=f"hin{tag}")
                for f in range(2):
                    nc.sync.dma_start(h_in[f], src_sb[:, f, :])
                h_all = dram.tile([N_CORES, 2, 128, BLK], BF16, tag=f"hall{tag}",
                                  addr_space="Local" if sim else "Shared")
                if sim:
                    for c in range(N_CORES):
                        nc.sync.dma_start(h_all[c], h_in[:])
                else:
                    nc.gpsimd.collective_compute(
                        "AllGather", mybir.AluOpType.bypass, replica_groups=GRP,
                        ins=[h_in.opt()], outs=[h_all.opt()])
                for c in range(N_CORES):
                    for f in range(2):
                        nc.scalar.dma_start(
                            hgat_sb[:, f, c * BLK:(c + 1) * BLK], h_all[c, f])

            def compute_hw(w_sb, n_out):
                with tc.tile_pool(name="hwpsum2", bufs=3, space="PSUM") as hp:
                    for s in range(NKC):
                        ps = hp.tile([128, n_out], F32, tag="hwps2")
                        nc.tensor.matmul(ps[:], hgat_sb[:, 0, bass.ts(s, 128)],
                                         w_sb[:, 0, :], start=True, stop=False)
                        nc.tensor.matmul(ps[:], hgat_sb[:, 1, bass.ts(s, 128)],
                                         w_sb[:, 1, :], start=False, stop=True)
                        if s % 2 == 0:
                            nc.vector.tensor_copy(hw_sb[:, s, 0:n_out], ps[:])
                        else:
                            nc.scalar.copy(hw_sb[:, s, 0:n_out], ps[:])

            with tc.tile_pool(name="apool", bufs=3) as apool, \
                 tc.tile_pool(name="gtmp", bufs=2) as gtmp, \
                 tc.tile_pool(name="hblk", bufs=2) as hblk:
                # ----- layer 1 -----
                with tc.tile_pool(name="gps1", bufs=1, space="PSUM") as gp1:
                    gcn_ps = gcn_accumulate(apool, gp1, HID, "a")
                    h1_sb = hblk.tile([128, 2, BLK], BF16, tag="hout")
                    gcn_finish_relu(gcn_ps, b1_sb, h1_sb, gtmp)
                allgather_h("1", h1_sb)
                compute_hw(w2_sb, HID)
                # ----- layer 2 -----
                with tc.tile_pool(name="gps2", bufs=1, space="PSUM") as gp2:
                    gcn_ps = gcn_accumulate(apool, gp2, HID, "b")
                    h2_sb = hblk.tile([128, 2, BLK], BF16, tag="hout")
                    gcn_finish_relu(gcn_ps, b2_sb, h2_sb, gtmp)
                allgather_h("2", h2_sb)
                compute_hw(w3_sb, OUT_DIM)
                # ----- layer 3 (no relu, f32 out) -----
                with tc.tile_pool(name="gps3", bufs=1, space="PSUM") as gp3:
                    ps3 = [gp3.tile([OUT_DIM, 512], F32, tag=f"g3{d}",
                                    name=f"gps3_{d}")
                           for d in range(2)]
                    ab = a_blk.rearrange("(g s p) d -> g p s d", s=4, p=128)
                    for g in range(NKC // 4):
                        a_t = apool.tile([128, 4, BLK], FP8, tag="achunk")
                        nc.sync.dma_start(a_t[:], ab[g])
                        for si in range(4):
                            s = g * 4 + si
                            for d in range(2):
                                nc.tensor.matmul(ps3[d][:], hw_sb[:, s, 0:OUT_DIM],
                                                 a_t[:, si, bass.ts(d, 512)],
                                                 start=(s == 0), stop=(s == NKC - 1))
                    o_sb = hblk.tile([OUT_DIM, BLK], F32, tag="osb", bufs=1)
                    for d in range(2):
                        t1 = gtmp.tile([OUT_DIM, 512], F32, tag="t3")
                        nc.vector.tensor_mul(t1[:], ps3[d][:],
                                             dinv_sb[0:OUT_DIM, bass.ts(d, 512)])
                        nc.scalar.activation(o_sb[:, bass.ts(d, 512)], t1[:],
                                             mybir.ActivationFunctionType.Identity,
                                             bias=b3_sb[:], scale=1.0)
                    nc.sync.dma_start(outT[:], o_sb[:])

        big_pool.release()
        const_pool.release()

    nc.compile()
    return nc


def _preprocess(x, edge_index, pe_w, pe_b, in_proj_w, in_proj_b,
                out_proj_w, out_proj_b, w1, b1, w2, b2, w3, b3):
    """Host-side sharding + weight folding. Returns per-core input maps."""
    x = _f32(x)
    src = np.asarray(edge_index[0], dtype=np.int64)
    dst = np.asarray(edge_index[1], dtype=np.int64)

    # G[src, dst] = edge multiplicity + self loops (small exact ints)
    G = np.zeros((N_NODES, N_NODES), dtype=np.float32)
    np.add.at(G, (src, dst), 1.0)
    idx = np.arange(N_NODES)
    G[idx, idx] += 1.0
    deg = G.sum(axis=0)
    dinv = (1.0 / np.sqrt(deg)).astype(np.float32)
    G8 = G.astype(NP_FP8)

    xT = _bf(x.T).reshape(2, 128, N_NODES)
    xsT = _bf((x * dinv[:, None]).T).reshape(2, 128, N_NODES)

    ipw = _f32(in_proj_w)
    ipb = _f32(in_proj_b)

    def aug_spread(w, b):  # [65, 128]: head h -> cols h*32 .. h*32+16
        out = np.zeros((PD + 1, 128), dtype=np.float32)
        for h in range(NH):
            out[0:PD, h * 32:h * 32 + DH] = w[h * DH:(h + 1) * DH].T
            out[PD, h * 32:h * 32 + DH] = b[h * DH:(h + 1) * DH]
        return _bf(out)

    wq_aug = aug_spread(ipw[0:PD] / 4.0, ipb[0:PD] / 4.0)
    wk_aug = aug_spread(ipw[PD:2 * PD], ipb[PD:2 * PD])
    wv = ipw[2 * PD:3 * PD]
    bv = ipb[2 * PD:3 * PD]
    # per head h (33 cols): cols 0..15 = wv_h.T (+bias row), 16..31 = 0,
    # col 32 = ones-row trick -> AV psum row 32 = softmax denominator
    wv_aug = np.zeros((PD + 1, NH * 33), dtype=np.float32)
    for h in range(NH):
        wv_aug[0:PD, h * 33:h * 33 + DH] = wv[h * DH:(h + 1) * DH].T
        wv_aug[PD, h * 33:h * 33 + DH] = bv[h * DH:(h + 1) * DH]
        wv_aug[PD, h * 33 + 32] = 1.0
    wv_aug = _bf(wv_aug)

    # wo spread: rows h*32..h*32+16 = out_proj_w.T rows h*16..h*16+16
    wo_sp = np.zeros((128, PD), dtype=np.float32)
    woT = np.asarray(out_proj_w, dtype=np.float32).T
    for h in range(NH):
        wo_sp[h * 32:h * 32 + DH, :] = woT[h * DH:(h + 1) * DH, :]
    wo_sp = _bf(wo_sp)

    # indicator: rb[f, q] = r_sp[32*(f//32), q] for data rows, 0 for pad rows
    ind128 = np.zeros((128, 128), dtype=np.float32)
    for f in range(128):
        if f % 32 < DH:
            ind128[(f // 32) * 32, f] = 1.0

    shared = {
        "xT": xT, "xsT": xsT,
        "pe_w": _bf(pe_w).reshape(2, 128, PD),
        "pe_b": _f32(pe_b).reshape(PD, 1),
        "wq_aug": wq_aug, "wk_aug": wk_aug, "wv_aug": wv_aug,
        "wo_sp": wo_sp,
        "bo": _f32(out_proj_b).reshape(PD, 1),
        "w1": _bf(w1), "b1": _f32(b1).reshape(2, 128).T.copy(),
        "w2": _bf(w2), "b2": _f32(b2).reshape(2, 128).T.copy(),
        "w3": _bf(w3), "b3": _f32(b3).reshape(OUT_DIM, 1),
        "ind128": ind128,
    }
    in_maps = []
    for c in range(N_CORES):
        lo, hi = c * BLK, (c + 1) * BLK
        m = dict(shared)
        m["xblkT"] = np.ascontiguousarray(
            xT.reshape(IN_DIM, N_NODES)[:, lo:hi]).reshape(2, 128, BLK)
        m["a_blk"] = np.ascontiguousarray(G8[:, lo:hi])
        m["dinv_b"] = np.ascontiguousarray(
            np.broadcast_to(dinv[lo:hi][None, :], (128, BLK)))
        in_maps.append(m)
    return in_maps


def kernel(**inputs):
    if "nc" not in _cache:
        _cache["nc"] = _build_program()
    nc = _cache["nc"]
    in_maps = _preprocess(**inputs)
    res = run_bass_kernel_spmd(nc, in_maps, list(range(N_CORES)))
    out = np.concatenate(
        [np.asarray(res.results[c]["outT"], dtype=np.float32).T
         for c in range(N_CORES)], axis=0)
    return out
